# revision 1
# baseline (speedup 1.0000x reference)
"""Cross-modal attention (B=4, C=512, L=2048, H=8, D=64) on 8 TRN2 NeuronCores.

Sharding: core c handles batch b = c//2 and query-half q = c%2 (1024 queries).
K/V are computed from the full ecg[b] on both cores of a pair (duplicated, no
collectives needed).

v3: the kernel is ACT-bound (softmax exp = 128 ACTIVATE x ~1.1us = 139us of
Scalar-engine time is the floor), so everything else is arranged to hide
under it:
  - fp8e4m3 DoubleRow matmuls (0.5 cycles/row) for the V/K/Q projections,
    the probs@V context matmul and the output projection; only the scores
    matmul (exp argument) and the residual stay bf16.  Weights are host-
    scaled by 16 into fp8's normal range; the 1/16 (resp. 1/256 for the
    doubly-scaled output projection) is folded into the bias add / the
    softmax-denominator scale, so no extra ops are spent.
  - software-pipelined emission: engines execute their streams in order,
    so ctx matmuls (which can block on the previous iteration's softmax
    normalization) are emitted LAG key-blocks behind the score/exp pair,
    and projection / out-projection work is injected into attention loops
    as 'extra' work that fills PE gaps while ACT churns.
  - single persistent PSUM pool: st(scores) 2 banks x2, ctx accumulators
    pc0/pc1 1 bank each, 'mm' (proj/out-proj) 1 bank x2.
  - out = residual + bias accumulated in SBUF f32; per-tile DMA out.
"""

import os
import numpy as np

B = 4
C = 512
L = 2048
H = 8
D = 64
LQ = 1024          # queries per core
P = 128
NCB = C // P       # 4 c-blocks (also head-pairs)
NKB = L // P       # 16 key blocks of 128
NG = NKB // 2      # 8 key groups of 256 (fp8 DoubleRow ctx)

_CACHED = {}


def _build():
    import concourse.tile as tile
    from concourse import bacc, mybir

    F32 = mybir.dt.float32
    CDT = mybir.dt.bfloat16
    F8 = mybir.dt.float8e4
    EXP = mybir.ActivationFunctionType.Exp
    DR = mybir.MatmulPerfMode.DoubleRow
    MUL = mybir.AluOpType.mult
    ADD = mybir.AluOpType.add

    nc = bacc.Bacc("TRN2", target_bir_lowering=False, debug=False)

    ppg_c8 = nc.dram_tensor("ppg_c8", (C, LQ), F8, kind="ExternalInput").ap()
    ppg_cb = nc.dram_tensor("ppg_cb", (C, LQ), CDT, kind="ExternalInput").ap()
    ecg_b8 = nc.dram_tensor("ecg_b8", (C, L), F8, kind="ExternalInput").ap()
    wqt8h = nc.dram_tensor("wqt8", (C, C), F8, kind="ExternalInput").ap()
    wkt8h = nc.dram_tensor("wkt8", (C, C), F8, kind="ExternalInput").ap()
    wvt8h = nc.dram_tensor("wvt8", (C, C), F8, kind="ExternalInput").ap()
    wot8h = nc.dram_tensor("wot8", (64, NCB, 2, C), F8,
                           kind="ExternalInput").ap()
    bq = nc.dram_tensor("bq", (C,), F32, kind="ExternalInput").ap()
    bk = nc.dram_tensor("bk", (C,), F32, kind="ExternalInput").ap()
    bv16 = nc.dram_tensor("bv16", (C,), F32, kind="ExternalInput").ap()
    bo = nc.dram_tensor("bo", (C,), F32, kind="ExternalInput").ap()
    outp = nc.dram_tensor("outp", (C, LQ), F32, kind="ExternalOutput").ap()

    with tile.TileContext(nc) as tc:
        with (
            tc.tile_pool(name="persist", bufs=1) as persist,
            tc.tile_pool(name="psum", bufs=1, space="PSUM") as psum,
            tc.tile_pool(name="exp_pool", bufs=6) as exp_pool,
            tc.tile_pool(name="sm_pool", bufs=2) as sm_pool,
        ):
            # ---- input DMAs, ordered by first use, spread over 3 queues ----
            ecg8 = persist.tile([P, NCB, L], F8)
            wkt8 = persist.tile([P, NCB, C], F8)
            wvt8 = persist.tile([P, NCB, C], F8)
            wqt8 = persist.tile([P, NCB, C], F8)
            ppg8 = persist.tile([P, NCB, LQ], F8)
            ppg_c = persist.tile([P, NCB, LQ], CDT)
            bq_t = persist.tile([P, NCB], F32)
            bk_t = persist.tile([P, NCB], F32)
            bo_t = persist.tile([P, NCB], F32)
            bv_row = persist.tile([1, C], CDT)
            wot8_t = persist.tile([64, NCB, 2, C], F8)
            ecg_hbm = ecg_b8.rearrange("(s p) l -> p s l", p=P)
            wkt_hbm = wkt8h.rearrange("(s p) o -> p s o", p=P)
            nc.gpsimd.dma_start(wkt8[:, :, 0:P], wkt_hbm[:, :, 0:P])
            nc.sync.dma_start(ecg8[:, 0, :], ecg_hbm[:, 0, :])
            nc.scalar.dma_start(ecg8[:, 2, :], ecg_hbm[:, 2, :])
            nc.sync.dma_start(ecg8[:, 1, :], ecg_hbm[:, 1, :])
            nc.gpsimd.dma_start(wvt8[:], wvt8h.rearrange("(s p) o -> p s o", p=P))
            nc.gpsimd.dma_start(ecg8[:, 3, :], ecg_hbm[:, 3, :])
            nc.gpsimd.dma_start(bv_row[0:1, :], bv16[None, :])
            nc.sync.dma_start(bk_t[:], bk.rearrange("(s p) -> p s", p=P))
            nc.sync.dma_start(bq_t[:], bq.rearrange("(s p) -> p s", p=P))
            nc.sync.dma_start(bo_t[:], bo.rearrange("(s p) -> p s", p=P))
            nc.scalar.dma_start(ppg8[:], ppg_c8.rearrange("(s p) l -> p s l", p=P))
            nc.gpsimd.dma_start(wkt8[:, :, P:], wkt_hbm[:, :, P:])
            nc.gpsimd.dma_start(wqt8[:], wqt8h.rearrange("(s p) o -> p s o", p=P))
            nc.sync.dma_start(ppg_c[:], ppg_cb.rearrange("(s p) l -> p s l", p=P))
            nc.gpsimd.dma_start(wot8_t[:], wot8h)

            ones_t = persist.tile([1, P], CDT)
            nc.vector.memset(ones_t[:], 1.0)
            ones_col = persist.tile([P, 1], F32)
            nc.vector.memset(ones_col[:], 1.0)

            # ---- persistent activations ----
            qT = persist.tile([P, NCB, LQ], CDT)
            kT = persist.tile([P, NCB, L], CDT)
            # v8: 16*(y@Wv^T+bv) in fp8, key-group-paired for DoubleRow,
            # padded M 65->80 (dual-fp8 ldweights needs 16B-aligned step),
            # ones column at D for the softmax denominator row.
            v8 = persist.tile([P, NG, 2, H, 80], F8)
            ctxT8 = persist.tile([64, NCB, 2, LQ], F8)   # 16*ctx/den
            out_acc = persist.tile([P, NCB, LQ], F32)
            nc.vector.tensor_copy(
                out=v8[:, :, :, :, D:D + 1],
                in_=ones_col[:, None, None, None, :].to_broadcast(
                    (P, NG, 2, H, 1)))

            def kT_chunk(cb, kb5):
                # kT[:, cb, 512-chunk] = (16*Wk @ y^T)/16 + bk
                ps_k = psum.tile([P, 512], F32, tag="mm", bufs=2)
                for s2 in range(2):
                    nc.tensor.matmul(
                        ps_k[:], wkt8[:, 2 * s2:2 * s2 + 2, cb * P:(cb + 1) * P],
                        ecg8[:, 2 * s2:2 * s2 + 2, kb5 * 512:(kb5 + 1) * 512],
                        start=(s2 == 0), stop=(s2 == 1), perf_mode=DR)
                nc.vector.tensor_scalar(
                    out=kT[:, cb, kb5 * 512:(kb5 + 1) * 512], in0=ps_k[:],
                    scalar1=1.0 / 16.0, scalar2=bk_t[:, cb:cb + 1],
                    op0=MUL, op1=ADD)

            def qT_chunk(cb, qb5):
                ps_q = psum.tile([P, 512], F32, tag="mm", bufs=2)
                for s2 in range(2):
                    nc.tensor.matmul(
                        ps_q[:], wqt8[:, 2 * s2:2 * s2 + 2, cb * P:(cb + 1) * P],
                        ppg8[:, 2 * s2:2 * s2 + 2, qb5 * 512:(qb5 + 1) * 512],
                        start=(s2 == 0), stop=(s2 == 1), perf_mode=DR)
                nc.vector.tensor_scalar(
                    out=qT[:, cb, qb5 * 512:(qb5 + 1) * 512], in0=ps_q[:],
                    scalar1=1.0 / 16.0, scalar2=bq_t[:, cb:cb + 1],
                    op0=MUL, op1=ADD)

            def v_block(lb):
                # v8[lb] = 16*(y[lb] @ Wv^T + bv)  (head-strided)
                ps_v = psum.tile([P, 512], F32, tag="mm", bufs=2)
                for s2 in range(2):
                    nc.tensor.matmul(
                        ps_v[:], ecg8[:, 2 * s2:2 * s2 + 2, lb * P:(lb + 1) * P],
                        wvt8[:, 2 * s2:2 * s2 + 2, :],
                        start=(s2 == 0), stop=False, perf_mode=DR)
                nc.tensor.matmul(ps_v[:], ones_t[0:1, :], bv_row[0:1, :],
                                 start=False, stop=True)
                nc.vector.tensor_copy(
                    out=v8[:, lb // 2, lb % 2, :, 0:D],
                    in_=ps_v[:].rearrange("p (h d) -> p h d", d=D))

            LAG = 6

            def attn(pair, qb, extra=None):
                # scores/exp per 128-key block; fp8 DoubleRow ctx per
                # 256-key group, emitted LAG blocks later so the (in-order)
                # PE stream never stalls ACT behind a norm-blocked ctx.
                qsl = slice(qb * 512, (qb + 1) * 512)
                pc0 = psum.tile([P, 512], F32, tag="pc0", bufs=1)
                pc1 = psum.tile([P, 512], F32, tag="pc1", bufs=1)
                pcs = (pc0, pc1)
                e8s = {}
                for kb in range(NKB + LAG):
                    if kb < NKB:
                        g, t = kb // 2, kb % 2
                        if t == 0:
                            e8s[g] = exp_pool.tile([P, 2, 2, 512], F8,
                                                   name="e8t", tag="e8",
                                                   bufs=6)
                        st = psum.tile([P, 2, 512], F32, tag="st", bufs=2)
                        for hl in range(2):
                            nc.tensor.matmul(
                                st[:, hl, :],
                                kT[64 * hl:64 * hl + 64, pair,
                                   kb * P:(kb + 1) * P],
                                qT[64 * hl:64 * hl + 64, pair, qsl],
                                start=True, stop=True)
                        nc.scalar.activation(e8s[g][:, t, :, :], st[:],
                                             EXP, scale=0.125)
                        if extra is not None and kb in extra:
                            extra[kb]()
                    j = kb - LAG
                    if j >= 1 and j % 2 == 1:
                        g = j // 2
                        for hl in range(2):
                            nc.tensor.matmul(
                                pcs[hl][0:D + 1, :],
                                v8[:, g, :, 2 * pair + hl, 0:D + 1],
                                e8s[g][:, :, hl, :],
                                start=(g == 0), stop=(g == NG - 1),
                                perf_mode=DR)
                        if g >= 1:
                            del e8s[g - 1]
                for hl in range(2):
                    den = sm_pool.tile([1, 512], F32)
                    nc.vector.tensor_copy(out=den[0:1, :],
                                          in_=pcs[hl][D:D + 1, :])
                    recip = sm_pool.tile([1, 512], F32)
                    nc.vector.reciprocal_approx_fast(
                        out=recip[0:1, :], in_=den[0:1, :])
                    rbc = sm_pool.tile([64, 512], F32)
                    nc.gpsimd.partition_broadcast(rbc[:], recip[0:1, :],
                                                  channels=64)
                    nc.vector.tensor_mul(
                        out=ctxT8[:, pair, hl, qsl], in0=pcs[hl][0:D, :],
                        in1=rbc[:])

            def po_pair(qb, cb, pair):
                # one DoubleRow matmul = this head-pair's contribution to
                # out[cb, qb]; accumulated straight into SBUF so no PSUM
                # bank is held across attention iterations.
                qsl = slice(qb * 512, (qb + 1) * 512)
                po = psum.tile([P, 512], F32, tag="mm", bufs=2)
                nc.tensor.matmul(
                    po[:], wot8_t[:, pair, :, cb * P:(cb + 1) * P],
                    ctxT8[:, pair, :, qsl],
                    start=True, stop=True, perf_mode=DR)
                # out_acc += po/256  (16*Wo and 16*ctx scaling)
                nc.vector.scalar_tensor_tensor(
                    out=out_acc[:, cb, qsl], in0=po[:],
                    scalar=1.0 / 256.0, in1=out_acc[:, cb, qsl],
                    op0=MUL, op1=ADD)
                if pair == H // 2 - 1:
                    nc.sync.dma_start(
                        outp.rearrange("(s p) l -> p s l", p=P)[:, cb, qsl],
                        out_acc[:, cb, qsl])

            # ---- pipelined emission ----
            kT_chunk(0, 0)
            qT_chunk(0, 0)
            v_block(0)
            # out_acc = residual + output bias (DVE fills gaps early)
            for cb in range(NCB):
                for qb in range(LQ // 512):
                    qsl = slice(qb * 512, (qb + 1) * 512)
                    nc.vector.tensor_scalar_add(
                        out_acc[:, cb, qsl], ppg_c[:, cb, qsl],
                        bo_t[:, cb:cb + 1])

            def merge(*exs):
                out = {}
                for ex in exs:
                    for k, fn in ex.items():
                        if k in out:
                            out[k] = (lambda a=out[k], b=fn: (a(), b()))
                        else:
                            out[k] = fn
                return out

            def proj_extras(cb):
                ex = {}
                for k5 in range(4):
                    ex[4 * k5] = (lambda c=cb, k=k5: kT_chunk(c, k))
                ex[2] = (lambda c=cb: qT_chunk(c, 0))
                ex[6] = (lambda c=cb: qT_chunk(c, 1))
                return ex

            def po_extras(qb, pair):
                # injected into the attention iteration FOLLOWING the norm
                # that produces ctxT8[pair, qb]; slots >= 7 so the in-order
                # PE stream never reaches them before the norm completes.
                return {7 + 2 * cb: (lambda q=qb, c=cb, p=pair: po_pair(q, c, p))
                        for cb in range(NCB)}

            ex00 = {kb: (lambda lb=kb + 1: v_block(lb))
                    for kb in range(NKB - 1)}
            ex00 = merge(ex00,
                         {0: (lambda: kT_chunk(0, 1)),
                          4: (lambda: kT_chunk(0, 2)),
                          8: (lambda: kT_chunk(0, 3)),
                          2: (lambda: qT_chunk(0, 1))})
            attn(0, 0, extra=ex00)
            attn(0, 1, extra=merge(proj_extras(1), po_extras(0, 0)))
            attn(1, 0, extra=po_extras(1, 0))
            attn(1, 1, extra=merge(proj_extras(2), po_extras(0, 1)))
            attn(2, 0, extra=po_extras(1, 1))
            attn(2, 1, extra=merge(proj_extras(3), po_extras(0, 2)))
            attn(3, 0, extra=po_extras(1, 2))
            attn(3, 1, extra=po_extras(0, 3))
            for cb in range(NCB):
                po_pair(1, cb, 3)
    nc.compile()
    return nc


def _get_nc():
    if "nc" not in _CACHED:
        _CACHED["nc"] = _build()
    return _CACHED["nc"]


def kernel(ppg, ecg, Wq, bq, Wk, bk, Wv, bv, Wo, bo):
    import ml_dtypes
    from concourse.bass_utils import run_bass_kernel_spmd

    nc = _get_nc()
    f = np.float32
    bf = ml_dtypes.bfloat16
    f8 = ml_dtypes.float8_e4m3fn
    wqt8 = np.ascontiguousarray((np.asarray(Wq, f).T * 16).astype(f8))
    wkt8 = np.ascontiguousarray((np.asarray(Wk, f).T * 16).astype(f8))
    wvt8 = np.ascontiguousarray((np.asarray(Wv, f).T * 16).astype(f8))
    # wot8[d, p, hl, o] = 16 * Wo[o, (2p+hl)*64 + d]
    wot8 = np.ascontiguousarray(
        (np.asarray(Wo, f).T * 16).reshape(NCB, 2, D, C)
        .transpose(2, 0, 1, 3).astype(f8))
    ppg = np.asarray(ppg, f)
    ecg = np.asarray(ecg, f)
    in_maps = []
    for c in range(8):
        b, half = c // 2, c % 2
        ppg_b = ppg[b][:, half * LQ:(half + 1) * LQ]
        in_maps.append({
            "ppg_c8": np.ascontiguousarray(ppg_b.astype(f8)),
            "ppg_cb": np.ascontiguousarray(ppg_b.astype(bf)),
            "ecg_b8": np.ascontiguousarray(ecg[b].astype(f8)),
            "wqt8": wqt8, "wkt8": wkt8, "wvt8": wvt8, "wot8": wot8,
            "bq": np.asarray(bq, f), "bk": np.asarray(bk, f),
            "bv16": np.asarray(bv, f) * 16, "bo": np.asarray(bo, f),
        })
    _CACHED["last_in_maps"] = in_maps
    res = run_bass_kernel_spmd(nc, in_maps, core_ids=list(range(8)))
    out = np.empty((B, C, L), f)
    for c, r in enumerate(res.results):
        b, half = c // 2, c % 2
        out[b][:, half * LQ:(half + 1) * LQ] = r["outp"]
    return out



# revision 10
# speedup vs baseline: 1.0334x; 1.0334x over previous
"""Cross-modal attention (B=4, C=512, L=2048, H=8, D=64) on 8 TRN2 NeuronCores.

Sharding: core c handles batch b = c//2 and query-half q = c%2 (1024 queries).
K/V are computed from the full ecg[b] on both cores of a pair (duplicated, no
collectives needed).

v4: ACT (softmax exp: 128 x ~1.07us = 137us) and PE (~135us visible) are
co-critical, and the HW power governor halves the PE clock whenever a 3.4us
window exceeds ~80% PE busy.  v4 therefore (a) cuts PE work and (b) flattens
the per-window PE duty:
  - v-projection bias via DVE broadcast-add instead of a ones-row matmul
    (kills 16 PE matmuls).
  - bk is dropped entirely: q.(k+bk) shifts every score of a query row by
    the same constant, which softmax cancels.  kT keeps the 16x fp8-weight
    scale (folded into the exp scale 0.125/16), saving the 1/16 rescale.
  - per-kb emission order is [extras, then scores]: the score matmul's
    ~173ns SBUF access latency hides behind the extras' drain instead of
    being paid on top each iteration.
  - ctx (probs@V, fp8 DoubleRow) for call i runs one call later, ~1 matmul
    per kb, so every call carries a near-constant PE load; the last two
    calls front-load their predecessors' ctx so only the final groups +
    norm + out-proj remain for the epilogue.
  - input DMAs are chunked critical-first across the sync/vector/gpsimd
    queues (never scalar: a dma_start costs 667ns of ACT sequencer time)
    so the first exp fires ~11us in; output DMAs stagger per 128x512 slab
    as the final out-projections complete.
  - single persistent PSUM pool: st (scores) 2 banks x2, pc (ctx acc)
    1 bank x2, mm (proj/out-proj) 1 bank x2.
"""

import os
import numpy as np

B = 4
C = 512
L = 2048
H = 8
D = 64
LQ = 1024          # queries per core
P = 128
NCB = C // P       # 4 c-blocks (also head-pairs)
NKB = L // P       # 16 key blocks of 128
NG = NKB // 2      # 8 key groups of 256 (fp8 DoubleRow ctx)

_CACHED = {}


def _build():
    import concourse.tile as tile
    from concourse import bacc, mybir

    F32 = mybir.dt.float32
    CDT = mybir.dt.bfloat16
    F8 = mybir.dt.float8e4
    EXP = mybir.ActivationFunctionType.Exp
    DR = mybir.MatmulPerfMode.DoubleRow
    MUL = mybir.AluOpType.mult
    ADD = mybir.AluOpType.add

    nc = bacc.Bacc("TRN2", target_bir_lowering=False, debug=False)

    ppg_c8 = nc.dram_tensor("ppg_c8", (C, LQ), F8, kind="ExternalInput").ap()
    ppg_cb = nc.dram_tensor("ppg_cb", (C, LQ), CDT, kind="ExternalInput").ap()
    ecg_b8 = nc.dram_tensor("ecg_b8", (C, L), F8, kind="ExternalInput").ap()
    wqt8h = nc.dram_tensor("wqt8", (C, C), F8, kind="ExternalInput").ap()
    wkt8h = nc.dram_tensor("wkt8", (C, C), F8, kind="ExternalInput").ap()
    wvt8h = nc.dram_tensor("wvt8", (C, C), F8, kind="ExternalInput").ap()
    wot8h = nc.dram_tensor("wot8", (64, NCB, 2, C), F8,
                           kind="ExternalInput").ap()
    bq = nc.dram_tensor("bq", (C,), F32, kind="ExternalInput").ap()
    bv16 = nc.dram_tensor("bv16", (C,), F32, kind="ExternalInput").ap()
    bo = nc.dram_tensor("bo", (C,), F32, kind="ExternalInput").ap()
    outp = nc.dram_tensor("outp", (C, LQ), F32, kind="ExternalOutput").ap()

    with tile.TileContext(nc) as tc:
        with (
            tc.tile_pool(name="persist", bufs=1) as persist,
            tc.tile_pool(name="psum", bufs=1, space="PSUM") as psum,
            tc.tile_pool(name="exp_pool", bufs=11) as exp_pool,
            tc.tile_pool(name="sm_pool", bufs=2) as sm_pool,
        ):
            # ---- persistent tiles ----
            ecg8 = persist.tile([P, NCB, L], F8)
            wkt8 = persist.tile([P, NCB, C], F8)
            wvt8 = persist.tile([P, NCB, C], F8)
            wqt8 = persist.tile([P, NCB, C], F8)
            ppg8 = persist.tile([P, NCB, LQ], F8)
            ppg_c = persist.tile([P, NCB, LQ], CDT)
            bq_t = persist.tile([P, NCB], F32)
            bo_t = persist.tile([P, NCB], F32)
            bv_row = persist.tile([1, C], F32)
            bv_bc = persist.tile([P, C], F32)
            wot8_t = persist.tile([64, NCB, 2, C], F8)

            # ---- input DMAs: critical-first chunks on 3 queues ----
            ecg_hbm = ecg_b8.rearrange("(s p) l -> p s l", p=P)
            wkt_hbm = wkt8h.rearrange("(s p) o -> p s o", p=P)
            wqt_hbm = wqt8h.rearrange("(s p) o -> p s o", p=P)
            ppg8_hbm = ppg_c8.rearrange("(s p) l -> p s l", p=P)

            # q_scalar: 3 critical loads only -- their 667ns dispatches run
            # during the framework preamble, before the first activation
            nc.scalar.dma_start(ecg8[:, 2, 0:512], ecg_hbm[:, 2, 0:512])
            nc.scalar.dma_start(ecg8[:, 3, 0:512], ecg_hbm[:, 3, 0:512])
            nc.scalar.dma_start(wqt8[:, :, 0:P], wqt_hbm[:, :, 0:P])
            # q_sync: bq, ecg s0/s1 l-chunk0, ppg8 first half
            nc.sync.dma_start(bq_t[:], bq.rearrange("(s p) -> p s", p=P))
            nc.sync.dma_start(ecg8[:, 0, 0:512], ecg_hbm[:, 0, 0:512])
            nc.sync.dma_start(ecg8[:, 1, 0:512], ecg_hbm[:, 1, 0:512])
            nc.sync.dma_start(ppg8[:, :, 0:512], ppg8_hbm[:, :, 0:512])
            # q_gpsimd: wkt cols0, wvt full (v blocks start early in call 0)
            nc.gpsimd.dma_start(wkt8[:, :, 0:P], wkt_hbm[:, :, 0:P])
            nc.gpsimd.dma_start(bv_row[0:1, :], bv16[None, :])
            nc.gpsimd.dma_start(wvt8[:], wvt8h.rearrange("(s p) o -> p s o", p=P))
            # second wave: ecg l-chunk1 first (kT(0,1) JIT at call-0 kb1)
            nc.sync.dma_start(ecg8[:, 0, 512:1024], ecg_hbm[:, 0, 512:1024])
            nc.sync.dma_start(ecg8[:, 1, 512:1024], ecg_hbm[:, 1, 512:1024])
            nc.gpsimd.dma_start(ecg8[:, 2, 512:1024], ecg_hbm[:, 2, 512:1024])
            nc.gpsimd.dma_start(ecg8[:, 3, 512:1024], ecg_hbm[:, 3, 512:1024])
            nc.sync.dma_start(ppg8[:, :, 512:1024], ppg8_hbm[:, :, 512:1024])
            nc.gpsimd.dma_start(wkt8[:, :, P:], wkt_hbm[:, :, P:])
            nc.sync.dma_start(ecg8[:, 0, 1024:2048], ecg_hbm[:, 0, 1024:2048])
            nc.sync.dma_start(ecg8[:, 1, 1024:2048], ecg_hbm[:, 1, 1024:2048])
            nc.gpsimd.dma_start(ecg8[:, 2, 1024:2048], ecg_hbm[:, 2, 1024:2048])
            nc.gpsimd.dma_start(ecg8[:, 3, 1024:2048], ecg_hbm[:, 3, 1024:2048])
            nc.gpsimd.dma_start(wqt8[:, :, P:], wqt_hbm[:, :, P:])
            nc.sync.dma_start(bo_t[:], bo.rearrange("(s p) -> p s", p=P))
            nc.gpsimd.dma_start(wot8_t[:], wot8h)
            # residual (needed by out_acc init before first po in call 2)
            nc.sync.dma_start(ppg_c[:], ppg_cb.rearrange("(s p) l -> p s l", p=P))

            nc.gpsimd.partition_broadcast(bv_bc[:], bv_row[0:1, :], channels=P)

            ones_col = persist.tile([P, 1], F32)
            nc.vector.memset(ones_col[:], 1.0)

            # ---- persistent activations ----
            qT = persist.tile([P, NCB, LQ], CDT)
            kT = persist.tile([P, NCB, L], CDT)   # holds 16*Wk@y (bk dropped)
            # v8: 16*(y@Wv^T+bv) in fp8, key-group-paired for DoubleRow,
            # padded M 65->80 (dual-fp8 ldweights needs 16B-aligned step),
            # ones column at D for the softmax denominator row.
            v8 = persist.tile([P, NG, 2, H, 80], F8)
            ctxT8 = persist.tile([64, NCB, 2, LQ], F8)   # 16*ctx/den
            out_acc = persist.tile([P, NCB, LQ], F32)
            nc.vector.tensor_copy(
                out=v8[:, :, :, :, D:D + 1],
                in_=ones_col[:, None, None, None, :].to_broadcast(
                    (P, NG, 2, H, 1)))

            def kT_chunk(cb, kb5):
                # kT[:, cb, 512-chunk] = 16*Wk @ y^T  (16x stays; exp scale
                # absorbs it; bk cancels in softmax)
                ps_k = psum.tile([P, 512], F32, tag="mm", bufs=2)
                for s2 in range(2):
                    nc.tensor.matmul(
                        ps_k[:], wkt8[:, 2 * s2:2 * s2 + 2, cb * P:(cb + 1) * P],
                        ecg8[:, 2 * s2:2 * s2 + 2, kb5 * 512:(kb5 + 1) * 512],
                        start=(s2 == 0), stop=(s2 == 1), perf_mode=DR)
                nc.vector.tensor_copy(
                    out=kT[:, cb, kb5 * 512:(kb5 + 1) * 512], in_=ps_k[:])

            def qT_chunk(cb, qb5):
                ps_q = psum.tile([P, 512], F32, tag="mm", bufs=2)
                for s2 in range(2):
                    nc.tensor.matmul(
                        ps_q[:], wqt8[:, 2 * s2:2 * s2 + 2, cb * P:(cb + 1) * P],
                        ppg8[:, 2 * s2:2 * s2 + 2, qb5 * 512:(qb5 + 1) * 512],
                        start=(s2 == 0), stop=(s2 == 1), perf_mode=DR)
                nc.vector.tensor_scalar(
                    out=qT[:, cb, qb5 * 512:(qb5 + 1) * 512], in0=ps_q[:],
                    scalar1=1.0 / 16.0, scalar2=bq_t[:, cb:cb + 1],
                    op0=MUL, op1=ADD)

            def v_block(lb):
                # v8[lb] = 16*(y[lb] @ Wv^T + bv)  (head-strided); bias via
                # DVE broadcast add (no PE matmul)
                ps_v = psum.tile([P, 512], F32, tag="mm", bufs=2)
                for s2 in range(2):
                    nc.tensor.matmul(
                        ps_v[:], ecg8[:, 2 * s2:2 * s2 + 2, lb * P:(lb + 1) * P],
                        wvt8[:, 2 * s2:2 * s2 + 2, :],
                        start=(s2 == 0), stop=(s2 == 1), perf_mode=DR)
                nc.vector.tensor_tensor(
                    out=v8[:, lb // 2, lb % 2, :, 0:D],
                    in0=ps_v[:].rearrange("p (h d) -> p h d", d=D),
                    in1=bv_bc[:].rearrange("p (h d) -> p h d", d=D),
                    op=ADD)

            def out_init(cb, qb):
                qsl = slice(qb * 512, (qb + 1) * 512)
                nc.vector.tensor_scalar_add(
                    out_acc[:, cb, qsl], ppg_c[:, cb, qsl],
                    bo_t[:, cb:cb + 1])

            # ---- attention machinery ----
            e8s = {}   # (pair, qb, g) -> tile; lives one full call
            pcs = {}   # (pair, qb) -> [pc_hl0, pc_hl1]

            def scores_kb(pair, qb, kb):
                qsl = slice(qb * 512, (qb + 1) * 512)
                g, t = kb // 2, kb % 2
                if t == 0:
                    e8s[(pair, qb, g)] = exp_pool.tile(
                        [P, 2, 2, 512], F8, name="e8t", tag="e8", bufs=11)
                st = psum.tile([P, 2, 512], F32, tag="st", bufs=2)
                for hl in range(2):
                    nc.tensor.matmul(
                        st[:, hl, :],
                        kT[64 * hl:64 * hl + 64, pair, kb * P:(kb + 1) * P],
                        qT[64 * hl:64 * hl + 64, pair, qsl],
                        start=True, stop=True)
                nc.scalar.activation(e8s[(pair, qb, g)][:, t, :, :], st[:],
                                     EXP, scale=0.125 / 16.0)

            def ctx_mm(pair, qb, g, hl):
                # one fp8-DR context matmul: pc[hl] += v8[g,:,head] @ e8
                if g == 0 and hl == 0:
                    pcs[(pair, qb)] = [
                        psum.tile([P, 512], F32, tag="pc", bufs=2, name="pc0"),
                        psum.tile([P, 512], F32, tag="pc", bufs=2, name="pc1")]
                pc = pcs[(pair, qb)][hl]
                nc.tensor.matmul(
                    pc[0:D + 1, :],
                    v8[:, g, :, 2 * pair + hl, 0:D + 1],
                    e8s[(pair, qb, g)][:, :, hl, :],
                    start=(g == 0), stop=(g == NG - 1),
                    perf_mode=DR)
                if hl == 1:
                    del e8s[(pair, qb, g)]

            def norm(pair, qb):
                # 1/den broadcast-mul; recip reads the PSUM den row directly
                qsl = slice(qb * 512, (qb + 1) * 512)
                for hl in range(2):
                    pc = pcs[(pair, qb)][hl]
                    den = sm_pool.tile([1, 512], F32)
                    nc.vector.tensor_copy(out=den[0:1, :],
                                          in_=pc[D:D + 1, :])
                    recip = sm_pool.tile([1, 512], F32)
                    nc.vector.reciprocal_approx_fast(
                        out=recip[0:1, :], in_=den[0:1, :])
                    rbc = sm_pool.tile([64, 512], F32)
                    nc.gpsimd.partition_broadcast(rbc[:], recip[0:1, :],
                                                  channels=64)
                    nc.vector.tensor_mul(
                        out=ctxT8[:, pair, hl, qsl], in0=pc[0:D, :],
                        in1=rbc[:])
                del pcs[(pair, qb)]

            def po_cb(pair, qb, cb, dma=False):
                # this head-pair's contribution to out[cb, qb-half]
                qsl = slice(qb * 512, (qb + 1) * 512)
                po = psum.tile([P, 512], F32, tag="mm", bufs=2)
                nc.tensor.matmul(
                    po[:], wot8_t[:, pair, :, cb * P:(cb + 1) * P],
                    ctxT8[:, pair, :, qsl],
                    start=True, stop=True, perf_mode=DR)
                nc.vector.scalar_tensor_tensor(
                    out=out_acc[:, cb, qsl], in0=po[:],
                    scalar=1.0 / 256.0, in1=out_acc[:, cb, qsl],
                    op0=MUL, op1=ADD)
                if dma:
                    q = (nc.sync, nc.gpsimd, nc.sync, nc.gpsimd)[cb]
                    q.dma_start(
                        outp.rearrange("(s p) l -> p s l", p=P)[:, cb, qsl],
                        out_acc[:, cb, qsl])

            def attn(pair, qb, extra, extras_first=True):
                # per kb: extras first (they fill the st-ring wait and the
                # score matmul's SBUF-access latency hides in their drain).
                # Call 0 uses extras_first=False: its extras have DMA-racy
                # deps that must not block the score feed in the in-order
                # PE queue.
                for kb in range(NKB):
                    if extras_first and kb in extra:
                        extra[kb]()
                    scores_kb(pair, qb, kb)
                    if not extras_first and kb in extra:
                        extra[kb]()

            def merge(*exs):
                out = {}
                for ex in exs:
                    for k, fn in ex.items():
                        if k in out:
                            out[k] = (lambda a=out[k], b=fn: (a(), b()))
                        else:
                            out[k] = fn
                return out

            def ctx_lagged(pair, qb):
                # prev call's ctx: singles kb3..12, doubles kb13..15
                # (kb0..2 left free so the previous norm's DVE chain and the
                # pc-ring WAR wait never block the PE stream)
                ex = {kb: (lambda p=pair, q=qb, i=kb - 3:
                           ctx_mm(p, q, i // 2, i % 2))
                      for kb in range(3, 13)}
                for kb in (13, 14, 15):
                    g = kb - 8
                    ex[kb] = (lambda p=pair, q=qb, g=g:
                              (ctx_mm(p, q, g, 0), ctx_mm(p, q, g, 1)))
                return ex

            def ctx_front(pair, qb, k0):
                # prev call's ctx at 2/kb in kb k0..k0+7 (tail calls)
                return {kb: (lambda p=pair, q=qb, g=kb - k0:
                             (ctx_mm(p, q, g, 0), ctx_mm(p, q, g, 1)))
                        for kb in range(k0, k0 + NG)}

            def po_call(pair, qb, kbs, dma=False):
                return {kbs[cb]: (lambda p=pair, q=qb, c=cb, d=dma:
                                  po_cb(p, q, c, d))
                        for cb in range(NCB)}

            # ---- prologue: minimal critical path to the first exp ----
            kT_chunk(0, 0)
            qT_chunk(0, 0)

            # ---- call 0: (0,0) + kT(0) JIT + qT(0,1) + v0..v11 ----
            vkb = (2, 3, 4, 6, 7, 8, 10, 11, 12, 14, 15)
            ex = {k: (lambda lb=i: v_block(lb)) for i, k in enumerate(vkb)}
            ex = merge(ex,
                       {6: lambda: v_block(11)},
                       {1: lambda: kT_chunk(0, 1),
                        5: lambda: kT_chunk(0, 2),
                        9: lambda: kT_chunk(0, 3),
                        13: lambda: qT_chunk(0, 1)})
            # call-0 v order: v0@2 v1@3 v2@4 v3@6 v11@6 v4@7 v5@8 v6@10
            #                 v7@11 v8@12 v9@14 v10@15
            attn(0, 0, ex, extras_first=False)

            # ---- call 1: (0,1); ctx(call0), v tail, kT(1,0..1), qT(1,0) ----
            ex = ctx_lagged(0, 0)
            ex = merge(ex, {1: lambda: v_block(13),
                            2: lambda: v_block(14),
                            4: lambda: v_block(15),
                            5: lambda: v_block(12),
                            0: lambda: kT_chunk(1, 0),
                            6: lambda: kT_chunk(1, 1),
                            12: lambda: qT_chunk(1, 0)})
            attn(0, 1, ex)
            norm(0, 0)

            # ---- call 2: (1,0); ctx(call1), po(call0), kT(1) rest ----
            ex = ctx_lagged(0, 1)
            ex = merge(ex, po_call(0, 0, (2, 5, 8, 11)),
                       {0: lambda: kT_chunk(1, 2),
                        1: lambda: kT_chunk(1, 3),
                        12: lambda: qT_chunk(1, 1)},
                       {1: lambda: out_init(0, 0), 3: lambda: out_init(1, 0),
                        4: lambda: out_init(2, 0), 6: lambda: out_init(3, 0),
                        7: lambda: out_init(0, 1), 9: lambda: out_init(1, 1),
                        10: lambda: out_init(2, 1),
                        13: lambda: out_init(3, 1)})
            attn(1, 0, ex)
            norm(0, 1)

            # ---- call 3: (1,1); ctx(call2), po(call1) ----
            ex = ctx_lagged(1, 0)
            ex = merge(ex, po_call(0, 1, (2, 5, 8, 11)),
                       {0: lambda: kT_chunk(2, 0),
                        1: lambda: kT_chunk(2, 1),
                        12: lambda: kT_chunk(2, 2),
                        14: lambda: qT_chunk(2, 0)})
            attn(1, 1, ex)
            norm(1, 0)

            # ---- call 4: (2,0) ----
            ex = ctx_lagged(1, 1)
            ex = merge(ex, po_call(1, 0, (2, 5, 8, 11)),
                       {0: lambda: kT_chunk(2, 3),
                        1: lambda: kT_chunk(3, 0),
                        12: lambda: qT_chunk(2, 1)})
            attn(2, 0, ex)
            norm(1, 1)

            # ---- call 5: (2,1) ----
            ex = ctx_lagged(2, 0)
            ex = merge(ex, po_call(1, 1, (2, 5, 8, 11)),
                       {0: lambda: kT_chunk(3, 1),
                        1: lambda: kT_chunk(3, 2),
                        12: lambda: kT_chunk(3, 3),
                        14: lambda: qT_chunk(3, 0)})
            attn(2, 1, ex)
            norm(2, 0)

            # ---- call 6: (3,0) ----
            ex = ctx_lagged(2, 1)
            ex = merge(ex, po_call(2, 0, (2, 5, 8, 11)),
                       {0: lambda: qT_chunk(3, 1)})
            attn(3, 0, ex)
            norm(2, 1)

            # ---- call 7: (3,1); ctx(call6) front at 2/kb, po(call5) after
            #      norm(2,1) clears, own ctx g0..3 lagged, po(call6)+qb0
            #      out DMAs in-call ----
            ex = ctx_front(3, 0, 2)
            ex = merge(ex, {10: lambda: norm(3, 0)},
                       {10: lambda: (po_cb(2, 1, 0), po_cb(2, 1, 1)),
                        11: lambda: (po_cb(2, 1, 2), po_cb(2, 1, 3))},
                       po_call(3, 0, (12, 13, 14, 15), dma=True),
                       {kb: (lambda g=kb - 12: (ctx_mm(3, 1, g, 0),
                                                ctx_mm(3, 1, g, 1)))
                        for kb in range(12, 16)})
            attn(3, 1, ex)

            # ---- epilogue: last ctx groups, norm, final po + out DMAs ----
            for g in (4, 5, 6, 7):
                ctx_mm(3, 1, g, 0)
                ctx_mm(3, 1, g, 1)
            norm(3, 1)
            for cb in range(NCB):
                po_cb(3, 1, cb, dma=True)
    nc.compile()
    return nc


def _get_nc():
    if "nc" not in _CACHED:
        _CACHED["nc"] = _build()
    return _CACHED["nc"]


def kernel(ppg, ecg, Wq, bq, Wk, bk, Wv, bv, Wo, bo):
    import ml_dtypes
    from concourse.bass_utils import run_bass_kernel_spmd

    nc = _get_nc()
    f = np.float32
    bf = ml_dtypes.bfloat16
    f8 = ml_dtypes.float8_e4m3fn
    wqt8 = np.ascontiguousarray((np.asarray(Wq, f).T * 16).astype(f8))
    wkt8 = np.ascontiguousarray((np.asarray(Wk, f).T * 16).astype(f8))
    wvt8 = np.ascontiguousarray((np.asarray(Wv, f).T * 16).astype(f8))
    # wot8[d, p, hl, o] = 16 * Wo[o, (2p+hl)*64 + d]
    wot8 = np.ascontiguousarray(
        (np.asarray(Wo, f).T * 16).reshape(NCB, 2, D, C)
        .transpose(2, 0, 1, 3).astype(f8))
    ppg = np.asarray(ppg, f)
    ecg = np.asarray(ecg, f)
    in_maps = []
    for c in range(8):
        b, half = c // 2, c % 2
        ppg_b = ppg[b][:, half * LQ:(half + 1) * LQ]
        in_maps.append({
            "ppg_c8": np.ascontiguousarray(ppg_b.astype(f8)),
            "ppg_cb": np.ascontiguousarray(ppg_b.astype(bf)),
            "ecg_b8": np.ascontiguousarray(ecg[b].astype(f8)),
            "wqt8": wqt8, "wkt8": wkt8, "wvt8": wvt8, "wot8": wot8,
            "bq": np.asarray(bq, f),
            "bv16": np.asarray(bv, f) * 16, "bo": np.asarray(bo, f),
        })
    _CACHED["last_in_maps"] = in_maps
    res = run_bass_kernel_spmd(nc, in_maps, core_ids=list(range(8)))
    out = np.empty((B, C, L), f)
    for c, r in enumerate(res.results):
        b, half = c // 2, c % 2
        out[b][:, half * LQ:(half + 1) * LQ] = r["outp"]
    return out


# revision 17
# speedup vs baseline: 1.0839x; 1.0488x over previous
"""Cross-modal attention (B=4, C=512, L=2048, H=8, D=64) on 8 TRN2 NeuronCores.

Sharding: core c handles batch b = c//2 and query-half q = c%2 (1024 queries).
K/V are computed from the full ecg[b] on both cores of a pair (duplicated, no
collectives needed).

v4: ACT (softmax exp: 128 x ~1.07us = 137us) and PE (~135us visible) are
co-critical, and the HW power governor halves the PE clock whenever a 3.4us
window exceeds ~80% PE busy.  v4 therefore (a) cuts PE work and (b) flattens
the per-window PE duty:
  - v-projection bias via DVE broadcast-add instead of a ones-row matmul
    (kills 16 PE matmuls).
  - bk is dropped entirely: q.(k+bk) shifts every score of a query row by
    the same constant, which softmax cancels.  kT keeps the 16x fp8-weight
    scale (folded into the exp scale 0.125/16), saving the 1/16 rescale.
  - per-kb emission order is [extras, then scores]: the score matmul's
    ~173ns SBUF access latency hides behind the extras' drain instead of
    being paid on top each iteration.
  - ctx (probs@V, fp8 DoubleRow) for call i runs one call later, ~1 matmul
    per kb, so every call carries a near-constant PE load; the last two
    calls front-load their predecessors' ctx so only the final groups +
    norm + out-proj remain for the epilogue.
  - input DMAs are chunked critical-first across the sync/vector/gpsimd
    queues (never scalar: a dma_start costs 667ns of ACT sequencer time)
    so the first exp fires ~11us in; output DMAs stagger per 128x512 slab
    as the final out-projections complete.
  - single persistent PSUM pool: st (scores) 2 banks x2, pc (ctx acc)
    1 bank x2, mm (proj/out-proj) 1 bank x2.
"""

import os
import numpy as np

B = 4
C = 512
L = 2048
H = 8
D = 64
LQ = 1024          # queries per core
P = 128
NCB = C // P       # 4 c-blocks (also head-pairs)
NKB = L // P       # 16 key blocks of 128
NG = NKB // 2      # 8 key groups of 256 (fp8 DoubleRow ctx)

_CACHED = {}


def _build():
    import concourse.tile as tile
    from concourse import bacc, mybir

    F32 = mybir.dt.float32
    CDT = mybir.dt.bfloat16
    F8 = mybir.dt.float8e4
    EXP = mybir.ActivationFunctionType.Exp
    DR = mybir.MatmulPerfMode.DoubleRow
    MUL = mybir.AluOpType.mult
    ADD = mybir.AluOpType.add

    nc = bacc.Bacc("TRN2", target_bir_lowering=False, debug=False)

    ppg_c8 = nc.dram_tensor("ppg_c8", (C, LQ), F8, kind="ExternalInput").ap()
    ppg_cb = nc.dram_tensor("ppg_cb", (C, LQ), CDT, kind="ExternalInput").ap()
    ecg_b8 = nc.dram_tensor("ecg_b8", (C, L), F8, kind="ExternalInput").ap()
    wqt8h = nc.dram_tensor("wqt8", (C, C), F8, kind="ExternalInput").ap()
    wkt8h = nc.dram_tensor("wkt8", (C, C), F8, kind="ExternalInput").ap()
    wvt8h = nc.dram_tensor("wvt8", (C, C), F8, kind="ExternalInput").ap()
    wot8h = nc.dram_tensor("wot8", (P, NCB, 2, 2, P), F8,
                           kind="ExternalInput").ap()
    bq = nc.dram_tensor("bq", (C,), F32, kind="ExternalInput").ap()
    bv16 = nc.dram_tensor("bv16", (C,), F32, kind="ExternalInput").ap()
    bo = nc.dram_tensor("bo", (C,), F32, kind="ExternalInput").ap()
    outp = nc.dram_tensor("outp", (C, LQ), F32, kind="ExternalOutput").ap()

    with tile.TileContext(nc) as tc:
        with (
            tc.tile_pool(name="persist", bufs=1) as persist,
            tc.tile_pool(name="psum", bufs=1, space="PSUM") as psum,
            tc.tile_pool(name="exp_pool", bufs=11) as exp_pool,
            tc.tile_pool(name="sm_pool", bufs=2) as sm_pool,
        ):
            # ---- persistent tiles ----
            ecg8 = persist.tile([P, NCB, L], F8)
            wkt8 = persist.tile([P, NCB, C], F8)
            wvt8 = persist.tile([P, NCB, C], F8)
            wqt8 = persist.tile([P, NCB, C], F8)
            ppg8 = persist.tile([P, NCB, LQ], F8)
            ppg_c = persist.tile([P, NCB, LQ], CDT)
            bq_t = persist.tile([P, NCB], F32)
            bo_t = persist.tile([P, NCB], F32)
            bv_row = persist.tile([1, C], F32)
            bv_bc = persist.tile([P, C], F32)
            # wot8_t rows 0..63: cb-even out-proj weights; rows 64..127:
            # cb-odd -- the two po matmuls of a cb pair run concurrently on
            # disjoint PE row groups
            wot8_t = persist.tile([P, NCB, 2, 2, P], F8)

            # ---- input DMAs: critical-first chunks on 3 queues ----
            ecg_hbm = ecg_b8.rearrange("(s p) l -> p s l", p=P)
            wkt_hbm = wkt8h.rearrange("(s p) o -> p s o", p=P)
            wqt_hbm = wqt8h.rearrange("(s p) o -> p s o", p=P)
            ppg8_hbm = ppg_c8.rearrange("(s p) l -> p s l", p=P)

            # q_scalar: 3 critical loads only -- their 667ns dispatches run
            # during the framework preamble, before the first activation
            nc.scalar.dma_start(ecg8[:, 2, 0:512], ecg_hbm[:, 2, 0:512])
            nc.scalar.dma_start(ecg8[:, 3, 0:512], ecg_hbm[:, 3, 0:512])
            nc.scalar.dma_start(wqt8[:, :, 0:P], wqt_hbm[:, :, 0:P])
            # q_sync: bq, ecg s0/s1 l-chunk0, ppg8 first half
            nc.sync.dma_start(bq_t[:], bq.rearrange("(s p) -> p s", p=P))
            nc.sync.dma_start(ecg8[:, 0, 0:512], ecg_hbm[:, 0, 0:512])
            nc.sync.dma_start(ecg8[:, 1, 0:512], ecg_hbm[:, 1, 0:512])
            nc.sync.dma_start(ppg8[:, :, 0:512], ppg8_hbm[:, :, 0:512])
            # q_gpsimd: wkt cols0, wvt full (v blocks start early in call 0)
            nc.gpsimd.dma_start(wkt8[:, :, 0:P], wkt_hbm[:, :, 0:P])
            nc.gpsimd.dma_start(bv_row[0:1, :], bv16[None, :])
            nc.gpsimd.dma_start(wvt8[:], wvt8h.rearrange("(s p) o -> p s o", p=P))
            # second wave: ecg l-chunk1 first (kT(0,1) JIT at call-0 kb1)
            nc.sync.dma_start(ecg8[:, 0, 512:1024], ecg_hbm[:, 0, 512:1024])
            nc.sync.dma_start(ecg8[:, 1, 512:1024], ecg_hbm[:, 1, 512:1024])
            nc.gpsimd.dma_start(ecg8[:, 2, 512:1024], ecg_hbm[:, 2, 512:1024])
            nc.gpsimd.dma_start(ecg8[:, 3, 512:1024], ecg_hbm[:, 3, 512:1024])
            nc.sync.dma_start(ppg8[:, :, 512:1024], ppg8_hbm[:, :, 512:1024])
            nc.gpsimd.dma_start(wkt8[:, :, P:], wkt_hbm[:, :, P:])
            nc.sync.dma_start(ecg8[:, 0, 1024:2048], ecg_hbm[:, 0, 1024:2048])
            nc.sync.dma_start(ecg8[:, 1, 1024:2048], ecg_hbm[:, 1, 1024:2048])
            nc.gpsimd.dma_start(ecg8[:, 2, 1024:2048], ecg_hbm[:, 2, 1024:2048])
            nc.gpsimd.dma_start(ecg8[:, 3, 1024:2048], ecg_hbm[:, 3, 1024:2048])
            nc.gpsimd.dma_start(wqt8[:, :, P:], wqt_hbm[:, :, P:])
            nc.sync.dma_start(bo_t[:], bo.rearrange("(s p) -> p s", p=P))
            nc.gpsimd.dma_start(wot8_t[:], wot8h)
            # residual (needed by out_acc init before first po in call 2)
            nc.sync.dma_start(ppg_c[:], ppg_cb.rearrange("(s p) l -> p s l", p=P))

            nc.gpsimd.partition_broadcast(bv_bc[:], bv_row[0:1, :], channels=P)

            ones_col = persist.tile([P, 1], F32)
            nc.vector.memset(ones_col[:], 1.0)

            # ---- persistent activations ----
            qT = persist.tile([P, NCB, LQ], CDT)
            kT = persist.tile([P, NCB, L], CDT)   # holds 16*Wk@y (bk dropped)
            # v8: 16*(y@Wv^T+bv) in fp8, key-group-paired for DoubleRow,
            # padded M 65->80 (dual-fp8 ldweights needs 16B-aligned step),
            # ones column at D for the softmax denominator row.
            v8 = persist.tile([P, NG, 2, H, 80], F8)
            # ctxT8 rows 0..63 written by norm; rows 64..127 are a DMA copy
            # for the cb-odd po matmuls' moving operand (row group 64)
            ctxT8 = persist.tile([P, NCB, 2, LQ], F8)   # 16*ctx/den
            out_acc = persist.tile([P, NCB, LQ], F32)
            nc.vector.tensor_copy(
                out=v8[:, :, :, :, D:D + 1],
                in_=ones_col[:, None, None, None, :].to_broadcast(
                    (P, NG, 2, H, 1)))

            def kT_chunk(cb, kb5):
                # kT[:, cb, 512-chunk] = 16*Wk @ y^T  (16x stays; exp scale
                # absorbs it; bk cancels in softmax)
                ps_k = psum.tile([P, 512], F32, tag="mm", bufs=2)
                for s2 in range(2):
                    nc.tensor.matmul(
                        ps_k[:], wkt8[:, 2 * s2:2 * s2 + 2, cb * P:(cb + 1) * P],
                        ecg8[:, 2 * s2:2 * s2 + 2, kb5 * 512:(kb5 + 1) * 512],
                        start=(s2 == 0), stop=(s2 == 1), perf_mode=DR)
                nc.vector.tensor_copy(
                    out=kT[:, cb, kb5 * 512:(kb5 + 1) * 512], in_=ps_k[:])

            def qT_chunk(cb, qb5):
                ps_q = psum.tile([P, 512], F32, tag="mm", bufs=2)
                for s2 in range(2):
                    nc.tensor.matmul(
                        ps_q[:], wqt8[:, 2 * s2:2 * s2 + 2, cb * P:(cb + 1) * P],
                        ppg8[:, 2 * s2:2 * s2 + 2, qb5 * 512:(qb5 + 1) * 512],
                        start=(s2 == 0), stop=(s2 == 1), perf_mode=DR)
                nc.vector.tensor_scalar(
                    out=qT[:, cb, qb5 * 512:(qb5 + 1) * 512], in0=ps_q[:],
                    scalar1=1.0 / 16.0, scalar2=bq_t[:, cb:cb + 1],
                    op0=MUL, op1=ADD)

            def v_block(lb):
                # v8[lb] = 16*(y[lb] @ Wv^T + bv)  (head-strided); bias via
                # DVE broadcast add (no PE matmul)
                ps_v = psum.tile([P, 512], F32, tag="mm", bufs=2)
                for s2 in range(2):
                    nc.tensor.matmul(
                        ps_v[:], ecg8[:, 2 * s2:2 * s2 + 2, lb * P:(lb + 1) * P],
                        wvt8[:, 2 * s2:2 * s2 + 2, :],
                        start=(s2 == 0), stop=(s2 == 1), perf_mode=DR)
                nc.vector.tensor_tensor(
                    out=v8[:, lb // 2, lb % 2, :, 0:D],
                    in0=ps_v[:].rearrange("p (h d) -> p h d", d=D),
                    in1=bv_bc[:].rearrange("p (h d) -> p h d", d=D),
                    op=ADD)

            def out_init(cb, qb):
                qsl = slice(qb * 512, (qb + 1) * 512)
                nc.vector.tensor_scalar_add(
                    out_acc[:, cb, qsl], ppg_c[:, cb, qsl],
                    bo_t[:, cb:cb + 1])

            # ---- attention machinery ----
            e8s = {}   # (pair, qb, g) -> tile; lives one full call
            pcs = {}   # (pair, qb) -> [pc_hl0, pc_hl1]

            def scores_kb(pair, qb, kb):
                qsl = slice(qb * 512, (qb + 1) * 512)
                g, t = kb // 2, kb % 2
                if t == 0:
                    e8s[(pair, qb, g)] = exp_pool.tile(
                        [P, 2, 2, 512], F8, name="e8t", tag="e8", bufs=11)
                st = psum.tile([P, 2, 512], F32, tag="st", bufs=2)
                for hl in range(2):
                    nc.tensor.matmul(
                        st[:, hl, :],
                        kT[64 * hl:64 * hl + 64, pair, kb * P:(kb + 1) * P],
                        qT[64 * hl:64 * hl + 64, pair, qsl],
                        start=True, stop=True)
                nc.scalar.activation(e8s[(pair, qb, g)][:, t, :, :], st[:],
                                     EXP, scale=0.125 / 16.0)

            def ctx_mm(pair, qb, g, hl):
                # one fp8-DR context matmul: pc[hl] += v8[g,:,head] @ e8
                if g == 0 and hl == 0:
                    pcs[(pair, qb)] = [
                        psum.tile([P, 512], F32, tag="pc", bufs=2, name="pc0"),
                        psum.tile([P, 512], F32, tag="pc", bufs=2, name="pc1")]
                pc = pcs[(pair, qb)][hl]
                nc.tensor.matmul(
                    pc[0:D + 1, :],
                    v8[:, g, :, 2 * pair + hl, 0:D + 1],
                    e8s[(pair, qb, g)][:, :, hl, :],
                    start=(g == 0), stop=(g == NG - 1),
                    perf_mode=DR)
                if hl == 1:
                    del e8s[(pair, qb, g)]

            def norm(pair, qb):
                qsl = slice(qb * 512, (qb + 1) * 512)
                for hl in range(2):
                    pc = pcs[(pair, qb)][hl]
                    den = sm_pool.tile([1, 512], F32)
                    nc.vector.tensor_copy(out=den[0:1, :],
                                          in_=pc[D:D + 1, :])
                    recip = sm_pool.tile([1, 512], F32)
                    nc.vector.reciprocal_approx_fast(
                        out=recip[0:1, :], in_=den[0:1, :])
                    rbc = sm_pool.tile([64, 512], F32)
                    nc.gpsimd.partition_broadcast(rbc[:], recip[0:1, :],
                                                  channels=64)
                    nc.vector.tensor_mul(
                        out=ctxT8[0:64, pair, hl, qsl], in0=pc[0:D, :],
                        in1=rbc[:])
                # duplicate to rows 64..127 for the cb-odd po row group
                nc.gpsimd.dma_start(ctxT8[64:P, pair, :, qsl],
                                    ctxT8[0:64, pair, :, qsl])
                del pcs[(pair, qb)]

            def po_pair(pair, qb, tp, dma=False):
                # cb=2tp on PE rows 0..63 and cb=2tp+1 on rows 64..127,
                # issued back-to-back so they overlap in the array
                qsl = slice(qb * 512, (qb + 1) * 512)
                pos = []
                for par in range(2):
                    cb = 2 * tp + par
                    po = psum.tile([P, 512], F32, tag="mm", bufs=2,
                                   name=f"po{par}")
                    r = slice(64 * par, 64 * par + 64)
                    nc.tensor.matmul(
                        po[:], wot8_t[r, pair, :, tp, :],
                        ctxT8[r, pair, :, qsl],
                        start=True, stop=True, perf_mode=DR)
                    pos.append(po)
                for par in range(2):
                    cb = 2 * tp + par
                    nc.vector.scalar_tensor_tensor(
                        out=out_acc[:, cb, qsl], in0=pos[par][:],
                        scalar=1.0 / 256.0, in1=out_acc[:, cb, qsl],
                        op0=MUL, op1=ADD)
                    if dma:
                        q = (nc.sync, nc.gpsimd)[par]
                        q.dma_start(
                            outp.rearrange("(s p) l -> p s l",
                                           p=P)[:, cb, qsl],
                            out_acc[:, cb, qsl])

            def attn(pair, qb, extra, extras_first=True):
                # per kb: extras first (they fill the st-ring wait and the
                # score matmul's SBUF-access latency hides in their drain).
                # Call 0 uses extras_first=False: its extras have DMA-racy
                # deps that must not block the score feed in the in-order
                # PE queue.
                for kb in range(NKB):
                    if extras_first and kb in extra:
                        extra[kb]()
                    scores_kb(pair, qb, kb)
                    if not extras_first and kb in extra:
                        extra[kb]()

            def merge(*exs):
                out = {}
                for ex in exs:
                    for k, fn in ex.items():
                        if k in out:
                            out[k] = (lambda a=out[k], b=fn: (a(), b()))
                        else:
                            out[k] = fn
                return out

            def ctx2(pair, qb, g):
                # 2-group chain: 4 back-to-back DR matmuls pipeline their
                # SBUF access latency (~243ns/mm vs ~310 solo)
                ctx_mm(pair, qb, g, 0)
                ctx_mm(pair, qb, g, 1)
                ctx_mm(pair, qb, g + 1, 0)
                ctx_mm(pair, qb, g + 1, 1)

            def ctx1(pair, qb, g):
                ctx_mm(pair, qb, g, 0)
                ctx_mm(pair, qb, g, 1)

            def ctx_chains(pair, qb, kbs=(3, 6, 9, 12)):
                # prev call's ctx as four 4-mm chains; kb0..2/14..15 stay
                # clear so the norm DVE chain and the pc-ring WAR wait
                # never block the call-boundary score feed
                return {kb: (lambda p=pair, q=qb, g=2 * i: ctx2(p, q, g))
                        for i, kb in enumerate(kbs)}

            # ---- prologue: minimal critical path to the first exp ----
            kT_chunk(0, 0)
            qT_chunk(0, 0)

            # ---- call 0: (0,0) + kT(0) JIT + qT(0,1) + v0..v7 ----
            ex = {2: lambda: (v_block(0), v_block(1)),
                  4: lambda: (v_block(2), v_block(3)),
                  6: lambda: (v_block(4), v_block(5)),
                  8: lambda: (v_block(6), v_block(7)),
                  1: lambda: kT_chunk(0, 1),
                  5: lambda: kT_chunk(0, 2),
                  10: lambda: kT_chunk(0, 3),
                  13: lambda: qT_chunk(0, 1)}
            attn(0, 0, ex, extras_first=False)

            # ---- call 1: (0,1); ctx(call0), v8..15, kT(1,0..1), qT(1,0) --
            ex = ctx_chains(0, 0)
            ex = merge(ex, {1: lambda: (v_block(8), v_block(9)),
                            4: lambda: (v_block(10), v_block(11)),
                            5: lambda: (v_block(12), v_block(13)),
                            7: lambda: (v_block(14), v_block(15)),
                            10: lambda: kT_chunk(1, 0),
                            13: lambda: qT_chunk(1, 0),
                            14: lambda: kT_chunk(1, 1)})
            attn(0, 1, ex)
            norm(0, 0)

            # ---- call 2: (1,0); ctx(call1), po(call0), kT(1) rest ----
            ex = ctx_chains(0, 1)
            ex = merge(ex, {5: lambda: po_pair(0, 0, 0),
                            11: lambda: po_pair(0, 0, 1),
                            4: lambda: kT_chunk(1, 2),
                            8: lambda: kT_chunk(1, 3),
                            13: lambda: qT_chunk(1, 1)},
                       {1: lambda: out_init(0, 0), 2: lambda: out_init(1, 0),
                        4: lambda: out_init(2, 0), 6: lambda: out_init(3, 0),
                        7: lambda: out_init(0, 1), 9: lambda: out_init(1, 1),
                        10: lambda: out_init(2, 1),
                        13: lambda: out_init(3, 1)})
            attn(1, 0, ex)
            norm(0, 1)

            # ---- call 3: (1,1); ctx(call2), po(call1) ----
            ex = ctx_chains(1, 0)
            ex = merge(ex, {5: lambda: po_pair(0, 1, 0),
                            11: lambda: po_pair(0, 1, 1),
                            4: lambda: kT_chunk(2, 0),
                            8: lambda: kT_chunk(2, 1),
                            13: lambda: kT_chunk(2, 2),
                            10: lambda: qT_chunk(2, 0)})
            attn(1, 1, ex)
            norm(1, 0)

            # ---- call 4: (2,0) ----
            ex = ctx_chains(1, 1)
            ex = merge(ex, {5: lambda: po_pair(1, 0, 0),
                            11: lambda: po_pair(1, 0, 1),
                            4: lambda: kT_chunk(2, 3),
                            8: lambda: kT_chunk(3, 0),
                            13: lambda: qT_chunk(2, 1)})
            attn(2, 0, ex)
            norm(1, 1)

            # ---- call 5: (2,1) ----
            ex = ctx_chains(2, 0)
            ex = merge(ex, {5: lambda: po_pair(1, 1, 0),
                            11: lambda: po_pair(1, 1, 1),
                            4: lambda: kT_chunk(3, 1),
                            8: lambda: kT_chunk(3, 2),
                            13: lambda: kT_chunk(3, 3),
                            10: lambda: qT_chunk(3, 0)})
            attn(2, 1, ex)
            norm(2, 0)

            # ---- call 6: (3,0); ctx(call5) front-shifted so norm(2,1) and
            #      the pc ring clear before call 7; own ctx g0..1 lag-run --
            ex = {2: lambda: ctx2(2, 1, 0),
                  4: lambda: ctx2(2, 1, 2),
                  6: lambda: ctx2(2, 1, 4),
                  8: lambda: ctx2(2, 1, 6),
                  9: lambda: norm(2, 1),
                  5: lambda: qT_chunk(3, 1),
                  10: lambda: po_pair(2, 0, 0),
                  12: lambda: po_pair(2, 0, 1),
                  11: lambda: ctx1(3, 0, 0),
                  13: lambda: ctx1(3, 0, 1)}
            attn(3, 0, ex)

            # ---- call 7: (3,1); rest of ctx(call6), po(2,1)+po(3,0),
            #      own ctx g0..4 lag-run, qb0 out DMAs in-call ----
            ex = {2: lambda: ctx2(3, 0, 2),
                  5: lambda: ctx2(3, 0, 4),
                  8: lambda: ctx2(3, 0, 6),
                  9: lambda: norm(3, 0),
                  6: lambda: po_pair(2, 1, 0),
                  10: lambda: po_pair(2, 1, 1),
                  12: lambda: (po_pair(3, 0, 0, dma=True), ctx1(3, 1, 0)),
                  13: lambda: ctx1(3, 1, 1),
                  14: lambda: (po_pair(3, 0, 1, dma=True), ctx1(3, 1, 2)),
                  15: lambda: (ctx1(3, 1, 3), ctx1(3, 1, 4))}
            attn(3, 1, ex)

            # ---- epilogue: last ctx groups, norm, final po + out DMAs ----
            ctx1(3, 1, 5)
            ctx1(3, 1, 6)
            ctx1(3, 1, 7)
            norm(3, 1)
            po_pair(3, 1, 0, dma=True)
            po_pair(3, 1, 1, dma=True)
    nc.compile()
    return nc


def _get_nc():
    if "nc" not in _CACHED:
        _CACHED["nc"] = _build()
    return _CACHED["nc"]


def kernel(ppg, ecg, Wq, bq, Wk, bk, Wv, bv, Wo, bo):
    import ml_dtypes
    from concourse.bass_utils import run_bass_kernel_spmd

    nc = _get_nc()
    f = np.float32
    bf = ml_dtypes.bfloat16
    f8 = ml_dtypes.float8_e4m3fn
    wqt8 = np.ascontiguousarray((np.asarray(Wq, f).T * 16).astype(f8))
    wkt8 = np.ascontiguousarray((np.asarray(Wk, f).T * 16).astype(f8))
    wvt8 = np.ascontiguousarray((np.asarray(Wv, f).T * 16).astype(f8))
    # wot8[par*64+d, p, hl, tp, j] = 16 * Wo[(2tp+par)*128 + j, (2p+hl)*64+d]
    # rows 0..63 hold cb-even weights, rows 64..127 cb-odd (po row groups)
    wot8 = np.ascontiguousarray(
        (np.asarray(Wo, f).T * 16)
        .reshape(NCB, 2, D, 2, 2, P)       # [p, hl, d, tp, par, j]
        .transpose(4, 2, 0, 1, 3, 5)       # [par, d, p, hl, tp, j]
        .reshape(P, NCB, 2, 2, P).astype(f8))
    ppg = np.asarray(ppg, f)
    ecg = np.asarray(ecg, f)
    in_maps = []
    for c in range(8):
        b, half = c // 2, c % 2
        ppg_b = ppg[b][:, half * LQ:(half + 1) * LQ]
        in_maps.append({
            "ppg_c8": np.ascontiguousarray(ppg_b.astype(f8)),
            "ppg_cb": np.ascontiguousarray(ppg_b.astype(bf)),
            "ecg_b8": np.ascontiguousarray(ecg[b].astype(f8)),
            "wqt8": wqt8, "wkt8": wkt8, "wvt8": wvt8, "wot8": wot8,
            "bq": np.asarray(bq, f),
            "bv16": np.asarray(bv, f) * 16, "bo": np.asarray(bo, f),
        })
    _CACHED["last_in_maps"] = in_maps
    res = run_bass_kernel_spmd(nc, in_maps, core_ids=list(range(8)))
    out = np.empty((B, C, L), f)
    for c, r in enumerate(res.results):
        b, half = c // 2, c % 2
        out[b][:, half * LQ:(half + 1) * LQ] = r["outp"]
    return out


# revision 28
# speedup vs baseline: 1.0950x; 1.0102x over previous
"""Cross-modal attention (B=4, C=512, L=2048, H=8, D=64) on 8 TRN2 NeuronCores.

Sharding: core c handles batch b = c//2 and query-half q = c%2 (1024 queries).
K/V are computed from the full ecg[b] on both cores of a pair (duplicated, no
collectives needed).

v4: ACT (softmax exp: 128 x ~1.07us = 137us) and PE (~135us visible) are
co-critical, and the HW power governor halves the PE clock whenever a 3.4us
window exceeds ~80% PE busy.  v4 therefore (a) cuts PE work and (b) flattens
the per-window PE duty:
  - v-projection bias via DVE broadcast-add instead of a ones-row matmul
    (kills 16 PE matmuls).
  - bk is dropped entirely: q.(k+bk) shifts every score of a query row by
    the same constant, which softmax cancels.  kT keeps the 16x fp8-weight
    scale (folded into the exp scale 0.125/16), saving the 1/16 rescale.
  - per-kb emission order is [extras, then scores]: the score matmul's
    ~173ns SBUF access latency hides behind the extras' drain instead of
    being paid on top each iteration.
  - ctx (probs@V, fp8 DoubleRow) for call i runs one call later, ~1 matmul
    per kb, so every call carries a near-constant PE load; the last two
    calls front-load their predecessors' ctx so only the final groups +
    norm + out-proj remain for the epilogue.
  - input DMAs are chunked critical-first across the sync/vector/gpsimd
    queues (never scalar: a dma_start costs 667ns of ACT sequencer time)
    so the first exp fires ~11us in; output DMAs stagger per 128x512 slab
    as the final out-projections complete.
  - single persistent PSUM pool: st (scores) 2 banks x2, pc (ctx acc)
    1 bank x2, mm (proj/out-proj) 1 bank x2.
"""

import os
import numpy as np

B = 4
C = 512
L = 2048
H = 8
D = 64
LQ = 1024          # queries per core
P = 128
NCB = C // P       # 4 c-blocks (also head-pairs)
NKB = L // P       # 16 key blocks of 128
NG = NKB // 2      # 8 key groups of 256 (fp8 DoubleRow ctx)

_CACHED = {}


def _build():
    import concourse.tile as tile
    from concourse import bacc, mybir

    F32 = mybir.dt.float32
    CDT = mybir.dt.bfloat16
    F8 = mybir.dt.float8e4
    EXP = mybir.ActivationFunctionType.Exp
    DR = mybir.MatmulPerfMode.DoubleRow
    MUL = mybir.AluOpType.mult
    ADD = mybir.AluOpType.add

    nc = bacc.Bacc("TRN2", target_bir_lowering=False, debug=False)

    ppg_c8 = nc.dram_tensor("ppg_c8", (C, LQ), F8, kind="ExternalInput").ap()
    ppg_cb = nc.dram_tensor("ppg_cb", (C, LQ), CDT, kind="ExternalInput").ap()
    ecg_b8 = nc.dram_tensor("ecg_b8", (C, L), F8, kind="ExternalInput").ap()
    wqt8h = nc.dram_tensor("wqt8", (C, C), F8, kind="ExternalInput").ap()
    wkt8h = nc.dram_tensor("wkt8", (C, C), F8, kind="ExternalInput").ap()
    wvt8h = nc.dram_tensor("wvt8", (C, C), F8, kind="ExternalInput").ap()
    wot8h = nc.dram_tensor("wot8", (P, NCB, 2, 2, P), F8,
                           kind="ExternalInput").ap()
    bq = nc.dram_tensor("bq", (C,), F32, kind="ExternalInput").ap()
    bv16 = nc.dram_tensor("bv16", (C,), F32, kind="ExternalInput").ap()
    bo = nc.dram_tensor("bo", (C,), F32, kind="ExternalInput").ap()
    outp = nc.dram_tensor("outp", (C, LQ), F32, kind="ExternalOutput").ap()

    with tile.TileContext(nc) as tc:
        with (
            tc.tile_pool(name="persist", bufs=1) as persist,
            tc.tile_pool(name="psum", bufs=1, space="PSUM") as psum,
            tc.tile_pool(name="exp_pool", bufs=11) as exp_pool,
            tc.tile_pool(name="sm_pool", bufs=2) as sm_pool,
        ):
            # ---- persistent tiles ----
            ecg8 = persist.tile([P, NCB, L], F8)
            wkt8 = persist.tile([P, NCB, C], F8)
            wvt8 = persist.tile([P, NCB, C], F8)
            wqt8 = persist.tile([P, NCB, C], F8)
            ppg8 = persist.tile([P, NCB, LQ], F8)
            ppg_c = persist.tile([P, NCB, LQ], CDT)
            bq_t = persist.tile([P, NCB], F32)
            bo_t = persist.tile([P, NCB], F32)
            bv_row = persist.tile([1, C], F32)
            bv_bc = persist.tile([P, C], F32)
            # wot8_t rows 0..63: cb-even out-proj weights; rows 64..127:
            # cb-odd -- the two po matmuls of a cb pair run concurrently on
            # disjoint PE row groups
            wot8_t = persist.tile([P, NCB, 2, 2, P], F8)

            # ---- input DMAs: critical-first chunks on 3 queues ----
            ecg_hbm = ecg_b8.rearrange("(s p) l -> p s l", p=P)
            wkt_hbm = wkt8h.rearrange("(s p) o -> p s o", p=P)
            wqt_hbm = wqt8h.rearrange("(s p) o -> p s o", p=P)
            ppg8_hbm = ppg_c8.rearrange("(s p) l -> p s l", p=P)

            # q_scalar: 3 critical loads only -- their 667ns dispatches run
            # during the framework preamble, before the first activation
            nc.scalar.dma_start(ecg8[:, 2, 0:512], ecg_hbm[:, 2, 0:512])
            nc.scalar.dma_start(ecg8[:, 3, 0:512], ecg_hbm[:, 3, 0:512])
            nc.scalar.dma_start(wqt8[:, :, 0:P], wqt_hbm[:, :, 0:P])
            # q_sync: bq, ecg s0/s1 l-chunk0, ppg8 first half
            nc.sync.dma_start(bq_t[:], bq.rearrange("(s p) -> p s", p=P))
            nc.sync.dma_start(ecg8[:, 0, 0:512], ecg_hbm[:, 0, 0:512])
            nc.sync.dma_start(ecg8[:, 1, 0:512], ecg_hbm[:, 1, 0:512])
            nc.sync.dma_start(ppg8[:, :, 0:512], ppg8_hbm[:, :, 0:512])
            # q_gpsimd: wkt cols0, wvt full (v blocks start early in call 0)
            nc.gpsimd.dma_start(wkt8[:, :, 0:P], wkt_hbm[:, :, 0:P])
            nc.gpsimd.dma_start(bv_row[0:1, :], bv16[None, :])
            nc.gpsimd.dma_start(wvt8[:], wvt8h.rearrange("(s p) o -> p s o", p=P))
            # second wave: ecg l-chunk1 first (kT(0,1) JIT at call-0 kb1)
            nc.sync.dma_start(ecg8[:, 0, 512:1024], ecg_hbm[:, 0, 512:1024])
            nc.sync.dma_start(ecg8[:, 1, 512:1024], ecg_hbm[:, 1, 512:1024])
            nc.gpsimd.dma_start(ecg8[:, 2, 512:1024], ecg_hbm[:, 2, 512:1024])
            nc.gpsimd.dma_start(ecg8[:, 3, 512:1024], ecg_hbm[:, 3, 512:1024])
            nc.sync.dma_start(ppg8[:, :, 512:1024], ppg8_hbm[:, :, 512:1024])
            nc.gpsimd.dma_start(wkt8[:, :, P:], wkt_hbm[:, :, P:])
            nc.sync.dma_start(ecg8[:, 0, 1024:2048], ecg_hbm[:, 0, 1024:2048])
            nc.sync.dma_start(ecg8[:, 1, 1024:2048], ecg_hbm[:, 1, 1024:2048])
            nc.gpsimd.dma_start(ecg8[:, 2, 1024:2048], ecg_hbm[:, 2, 1024:2048])
            nc.gpsimd.dma_start(ecg8[:, 3, 1024:2048], ecg_hbm[:, 3, 1024:2048])
            nc.gpsimd.dma_start(wqt8[:, :, P:], wqt_hbm[:, :, P:])
            nc.sync.dma_start(bo_t[:], bo.rearrange("(s p) -> p s", p=P))
            nc.gpsimd.dma_start(wot8_t[:], wot8h)
            # residual (needed by out_acc init before first po in call 2)
            nc.sync.dma_start(ppg_c[:], ppg_cb.rearrange("(s p) l -> p s l", p=P))

            nc.gpsimd.partition_broadcast(bv_bc[:], bv_row[0:1, :], channels=P)

            ones_col = persist.tile([P, 1], F32)
            nc.vector.memset(ones_col[:], 1.0)

            # ---- persistent activations ----
            qT = persist.tile([P, NCB, LQ], CDT)
            kT = persist.tile([P, NCB, L], CDT)   # holds 16*Wk@y (bk dropped)
            # v8: 16*(y@Wv^T+bv) in fp8, key-group-paired for DoubleRow,
            # padded M 65->80 (dual-fp8 ldweights needs 16B-aligned step),
            # ones column at D for the softmax denominator row.
            v8 = persist.tile([P, NG, 2, H, 80], F8)
            # ctxT8 rows 0..63 written by norm; rows 64..127 are a DMA copy
            # for the cb-odd po matmuls' moving operand (row group 64)
            ctxT8 = persist.tile([P, NCB, 2, LQ], F8)   # 16*ctx/den
            out_acc = persist.tile([P, NCB, LQ], F32)
            nc.vector.tensor_copy(
                out=v8[:, :, :, :, D:D + 1],
                in_=ones_col[:, None, None, None, :].to_broadcast(
                    (P, NG, 2, H, 1)))

            def kT_chunk(cb, kb5):
                # kT[:, cb, 512-chunk] = 16*Wk @ y^T  (16x stays; exp scale
                # absorbs it; bk cancels in softmax)
                ps_k = psum.tile([P, 512], F32, tag="mm", bufs=2)
                for s2 in range(2):
                    nc.tensor.matmul(
                        ps_k[:], wkt8[:, 2 * s2:2 * s2 + 2, cb * P:(cb + 1) * P],
                        ecg8[:, 2 * s2:2 * s2 + 2, kb5 * 512:(kb5 + 1) * 512],
                        start=(s2 == 0), stop=(s2 == 1), perf_mode=DR)
                nc.vector.tensor_copy(
                    out=kT[:, cb, kb5 * 512:(kb5 + 1) * 512], in_=ps_k[:])

            def qT_chunk(cb, qb5):
                ps_q = psum.tile([P, 512], F32, tag="mm", bufs=2)
                for s2 in range(2):
                    nc.tensor.matmul(
                        ps_q[:], wqt8[:, 2 * s2:2 * s2 + 2, cb * P:(cb + 1) * P],
                        ppg8[:, 2 * s2:2 * s2 + 2, qb5 * 512:(qb5 + 1) * 512],
                        start=(s2 == 0), stop=(s2 == 1), perf_mode=DR)
                nc.vector.tensor_scalar(
                    out=qT[:, cb, qb5 * 512:(qb5 + 1) * 512], in0=ps_q[:],
                    scalar1=1.0 / 16.0, scalar2=bq_t[:, cb:cb + 1],
                    op0=MUL, op1=ADD)

            def v_block(lb):
                # v8[lb] = 16*(y[lb] @ Wv^T + bv)  (head-strided); bias via
                # DVE broadcast add (no PE matmul)
                ps_v = psum.tile([P, 512], F32, tag="mm", bufs=2)
                for s2 in range(2):
                    nc.tensor.matmul(
                        ps_v[:], ecg8[:, 2 * s2:2 * s2 + 2, lb * P:(lb + 1) * P],
                        wvt8[:, 2 * s2:2 * s2 + 2, :],
                        start=(s2 == 0), stop=(s2 == 1), perf_mode=DR)
                nc.vector.tensor_tensor(
                    out=v8[:, lb // 2, lb % 2, :, 0:D],
                    in0=ps_v[:].rearrange("p (h d) -> p h d", d=D),
                    in1=bv_bc[:].rearrange("p (h d) -> p h d", d=D),
                    op=ADD)

            def out_init(cb, qb):
                qsl = slice(qb * 512, (qb + 1) * 512)
                nc.vector.tensor_scalar_add(
                    out_acc[:, cb, qsl], ppg_c[:, cb, qsl],
                    bo_t[:, cb:cb + 1])

            # ---- attention machinery ----
            e8s = {}   # (pair, qb, g) -> tile; lives one full call
            pcs = {}   # (pair, qb) -> [pc_hl0, pc_hl1]

            def scores_kb(pair, qb, kb):
                qsl = slice(qb * 512, (qb + 1) * 512)
                g, t = kb // 2, kb % 2
                if t == 0:
                    e8s[(pair, qb, g)] = exp_pool.tile(
                        [P, 2, 2, 512], F8, name="e8t", tag="e8", bufs=11)
                st = psum.tile([P, 2, 512], F32, tag="st", bufs=2)
                for hl in range(2):
                    nc.tensor.matmul(
                        st[:, hl, :],
                        kT[64 * hl:64 * hl + 64, pair, kb * P:(kb + 1) * P],
                        qT[64 * hl:64 * hl + 64, pair, qsl],
                        start=True, stop=True)
                nc.scalar.activation(e8s[(pair, qb, g)][:, t, :, :], st[:],
                                     EXP, scale=0.125 / 16.0)

            def ctx_mm(pair, qb, g, hl):
                # one fp8-DR context matmul: pc[hl] += v8[g,:,head] @ e8
                if g == 0 and hl == 0:
                    pcs[(pair, qb)] = [
                        psum.tile([P, 512], F32, tag="pc", bufs=2, name="pc0"),
                        psum.tile([P, 512], F32, tag="pc", bufs=2, name="pc1")]
                pc = pcs[(pair, qb)][hl]
                nc.tensor.matmul(
                    pc[0:D + 1, :],
                    v8[:, g, :, 2 * pair + hl, 0:D + 1],
                    e8s[(pair, qb, g)][:, :, hl, :],
                    start=(g == 0), stop=(g == NG - 1),
                    perf_mode=DR)
                if hl == 1:
                    del e8s[(pair, qb, g)]

            def norm(pair, qb):
                qsl = slice(qb * 512, (qb + 1) * 512)
                for hl in range(2):
                    pc = pcs[(pair, qb)][hl]
                    den = sm_pool.tile([1, 512], F32)
                    nc.vector.tensor_copy(out=den[0:1, :],
                                          in_=pc[D:D + 1, :])
                    recip = sm_pool.tile([1, 512], F32)
                    nc.vector.reciprocal_approx_fast(
                        out=recip[0:1, :], in_=den[0:1, :])
                    rbc = sm_pool.tile([64, 512], F32)
                    nc.gpsimd.partition_broadcast(rbc[:], recip[0:1, :],
                                                  channels=64)
                    nc.vector.tensor_mul(
                        out=ctxT8[0:64, pair, hl, qsl], in0=pc[0:D, :],
                        in1=rbc[:])
                # duplicate to rows 64..127 for the cb-odd po row group
                nc.gpsimd.dma_start(ctxT8[64:P, pair, :, qsl],
                                    ctxT8[0:64, pair, :, qsl])
                del pcs[(pair, qb)]

            def po_pair(pair, qb, tp, dma=False):
                # cb=2tp on PE rows 0..63 and cb=2tp+1 on rows 64..127,
                # issued back-to-back so they overlap in the array
                qsl = slice(qb * 512, (qb + 1) * 512)
                pos = []
                for par in range(2):
                    cb = 2 * tp + par
                    po = psum.tile([P, 512], F32, tag="mm", bufs=2,
                                   name=f"po{par}")
                    r = slice(64 * par, 64 * par + 64)
                    nc.tensor.matmul(
                        po[:], wot8_t[r, pair, :, tp, :],
                        ctxT8[r, pair, :, qsl],
                        start=True, stop=True, perf_mode=DR)
                    pos.append(po)
                outp_r = outp.rearrange("(s p) l -> p s l", p=P)
                for par in range(2):
                    cb = 2 * tp + par
                    nc.vector.scalar_tensor_tensor(
                        out=out_acc[:, cb, qsl], in0=pos[par][:],
                        scalar=1.0 / 256.0, in1=out_acc[:, cb, qsl],
                        op0=MUL, op1=ADD)
                    if dma == "split":
                        # tail: spread each slab over 3 queues (scalar is
                        # idle after the last exp)
                        h = qb * 512
                        nc.sync.dma_start(
                            outp_r[:, cb, h:h + 171],
                            out_acc[:, cb, h:h + 171])
                        nc.gpsimd.dma_start(
                            outp_r[:, cb, h + 171:h + 342],
                            out_acc[:, cb, h + 171:h + 342])
                        nc.scalar.dma_start(
                            outp_r[:, cb, h + 342:h + 512],
                            out_acc[:, cb, h + 342:h + 512])
                    elif dma:
                        q = (nc.sync, nc.gpsimd)[par]
                        q.dma_start(outp_r[:, cb, qsl],
                                    out_acc[:, cb, qsl])

            def attn(pair, qb, extra, extras_first=True):
                # per kb: extras first (they fill the st-ring wait and the
                # score matmul's SBUF-access latency hides in their drain).
                # Call 0 uses extras_first=False: its extras have DMA-racy
                # deps that must not block the score feed in the in-order
                # PE queue.
                for kb in range(NKB):
                    if extras_first and kb in extra:
                        extra[kb]()
                    scores_kb(pair, qb, kb)
                    if not extras_first and kb in extra:
                        extra[kb]()

            def merge(*exs):
                out = {}
                for ex in exs:
                    for k, fn in ex.items():
                        if k in out:
                            out[k] = (lambda a=out[k], b=fn: (a(), b()))
                        else:
                            out[k] = fn
                return out

            def ctx2(pair, qb, g):
                # 2-group chain: 4 back-to-back DR matmuls pipeline their
                # SBUF access latency (~243ns/mm vs ~310 solo)
                ctx_mm(pair, qb, g, 0)
                ctx_mm(pair, qb, g, 1)
                ctx_mm(pair, qb, g + 1, 0)
                ctx_mm(pair, qb, g + 1, 1)

            def ctx1(pair, qb, g):
                ctx_mm(pair, qb, g, 0)
                ctx_mm(pair, qb, g, 1)

            def ctx_chains(pair, qb, kbs=(3, 6, 9, 12)):
                # prev call's ctx as four 4-mm chains; kb0..2/14..15 stay
                # clear so the norm DVE chain and the pc-ring WAR wait
                # never block the call-boundary score feed
                return {kb: (lambda p=pair, q=qb, g=2 * i: ctx2(p, q, g))
                        for i, kb in enumerate(kbs)}

            # ---- prologue: minimal critical path to the first exp ----
            kT_chunk(0, 0)
            qT_chunk(0, 0)

            # ---- call 0: (0,0) + kT(0) JIT + qT(0,1) + v0..v7 ----
            ex = {2: lambda: (v_block(0), v_block(1)),
                  4: lambda: (v_block(2), v_block(3)),
                  6: lambda: (v_block(4), v_block(5)),
                  8: lambda: (v_block(6), v_block(7)),
                  1: lambda: kT_chunk(0, 1),
                  5: lambda: kT_chunk(0, 2),
                  10: lambda: kT_chunk(0, 3),
                  13: lambda: qT_chunk(0, 1)}
            attn(0, 0, ex, extras_first=False)

            # ---- call 1: (0,1); ctx(call0), v8..15, kT(1,0..1), qT(1,0) --
            ex = ctx_chains(0, 0)
            ex = merge(ex, {1: lambda: (v_block(8), v_block(9)),
                            4: lambda: (v_block(10), v_block(11)),
                            5: lambda: (v_block(12), v_block(13)),
                            7: lambda: (v_block(14), v_block(15)),
                            10: lambda: kT_chunk(1, 0),
                            13: lambda: qT_chunk(1, 0),
                            14: lambda: kT_chunk(1, 1)})
            attn(0, 1, ex)
            norm(0, 0)

            # ---- call 2: (1,0); ctx(call1), po(call0), kT(1) rest ----
            ex = ctx_chains(0, 1)
            ex = merge(ex, {5: lambda: po_pair(0, 0, 0),
                            11: lambda: po_pair(0, 0, 1),
                            4: lambda: kT_chunk(1, 2),
                            8: lambda: kT_chunk(1, 3),
                            13: lambda: qT_chunk(1, 1)},
                       {1: lambda: out_init(0, 0), 2: lambda: out_init(1, 0),
                        4: lambda: out_init(2, 0), 6: lambda: out_init(3, 0),
                        7: lambda: out_init(0, 1), 9: lambda: out_init(1, 1),
                        10: lambda: out_init(2, 1),
                        13: lambda: out_init(3, 1)})
            attn(1, 0, ex)
            norm(0, 1)

            # ---- call 3: (1,1); ctx(call2), po(call1) ----
            ex = ctx_chains(1, 0)
            ex = merge(ex, {5: lambda: po_pair(0, 1, 0),
                            11: lambda: po_pair(0, 1, 1),
                            4: lambda: kT_chunk(2, 0),
                            8: lambda: kT_chunk(2, 1),
                            13: lambda: kT_chunk(2, 2),
                            10: lambda: qT_chunk(2, 0)})
            attn(1, 1, ex)
            norm(1, 0)

            # ---- call 4: (2,0) ----
            ex = ctx_chains(1, 1)
            ex = merge(ex, {5: lambda: po_pair(1, 0, 0),
                            11: lambda: po_pair(1, 0, 1),
                            4: lambda: kT_chunk(2, 3),
                            8: lambda: kT_chunk(3, 0),
                            13: lambda: qT_chunk(2, 1)})
            attn(2, 0, ex)
            norm(1, 1)

            # ---- call 5: (2,1) ----
            ex = ctx_chains(2, 0)
            ex = merge(ex, {5: lambda: po_pair(1, 1, 0),
                            11: lambda: po_pair(1, 1, 1),
                            4: lambda: kT_chunk(3, 1),
                            8: lambda: kT_chunk(3, 2),
                            13: lambda: kT_chunk(3, 3),
                            10: lambda: qT_chunk(3, 0)})
            attn(2, 1, ex)
            norm(2, 0)

            # ---- call 6: (3,0); ctx(call5) front-shifted so norm(2,1) and
            #      the pc ring clear before call 7; own ctx g0..1 lag-run --
            ex = {2: lambda: ctx2(2, 1, 0),
                  4: lambda: ctx2(2, 1, 2),
                  6: lambda: ctx2(2, 1, 4),
                  8: lambda: ctx2(2, 1, 6),
                  9: lambda: norm(2, 1),
                  5: lambda: qT_chunk(3, 1),
                  10: lambda: po_pair(2, 0, 0),
                  12: lambda: po_pair(2, 0, 1),
                  11: lambda: ctx1(3, 0, 0),
                  13: lambda: ctx1(3, 0, 1)}
            attn(3, 0, ex)

            # ---- call 7: (3,1); rest of ctx(call6), po(2,1)+po(3,0),
            #      own ctx g0..4 lag-run, qb0 out DMAs in-call ----
            ex = {2: lambda: ctx2(3, 0, 2),
                  5: lambda: ctx2(3, 0, 4),
                  8: lambda: ctx2(3, 0, 6),
                  9: lambda: norm(3, 0),
                  10: lambda: po_pair(2, 1, 1),
                  6: lambda: po_pair(2, 1, 0),
                  12: lambda: (po_pair(3, 0, 0, dma=True), ctx1(3, 1, 0)),
                  13: lambda: (ctx1(3, 1, 1), ctx1(3, 1, 2)),
                  14: lambda: (po_pair(3, 0, 1, dma=True), ctx1(3, 1, 3)),
                  15: lambda: (ctx1(3, 1, 4), ctx1(3, 1, 5),
                               ctx1(3, 1, 6))}
            attn(3, 1, ex)

            # ---- epilogue: last ctx groups, norm, final po + out DMAs ----
            ctx1(3, 1, 7)
            norm(3, 1)
            po_pair(3, 1, 0, dma="split")
            po_pair(3, 1, 1, dma="split")
    nc.compile()
    return nc


def _get_nc():
    if "nc" not in _CACHED:
        _CACHED["nc"] = _build()
    return _CACHED["nc"]


def kernel(ppg, ecg, Wq, bq, Wk, bk, Wv, bv, Wo, bo):
    import ml_dtypes
    from concourse.bass_utils import run_bass_kernel_spmd

    nc = _get_nc()
    f = np.float32
    bf = ml_dtypes.bfloat16
    f8 = ml_dtypes.float8_e4m3fn
    wqt8 = np.ascontiguousarray((np.asarray(Wq, f).T * 16).astype(f8))
    wkt8 = np.ascontiguousarray((np.asarray(Wk, f).T * 16).astype(f8))
    wvt8 = np.ascontiguousarray((np.asarray(Wv, f).T * 16).astype(f8))
    # wot8[par*64+d, p, hl, tp, j] = 16 * Wo[(2tp+par)*128 + j, (2p+hl)*64+d]
    # rows 0..63 hold cb-even weights, rows 64..127 cb-odd (po row groups)
    wot8 = np.ascontiguousarray(
        (np.asarray(Wo, f).T * 16)
        .reshape(NCB, 2, D, 2, 2, P)       # [p, hl, d, tp, par, j]
        .transpose(4, 2, 0, 1, 3, 5)       # [par, d, p, hl, tp, j]
        .reshape(P, NCB, 2, 2, P).astype(f8))
    ppg = np.asarray(ppg, f)
    ecg = np.asarray(ecg, f)
    in_maps = []
    for c in range(8):
        b, half = c // 2, c % 2
        ppg_b = ppg[b][:, half * LQ:(half + 1) * LQ]
        in_maps.append({
            "ppg_c8": np.ascontiguousarray(ppg_b.astype(f8)),
            "ppg_cb": np.ascontiguousarray(ppg_b.astype(bf)),
            "ecg_b8": np.ascontiguousarray(ecg[b].astype(f8)),
            "wqt8": wqt8, "wkt8": wkt8, "wvt8": wvt8, "wot8": wot8,
            "bq": np.asarray(bq, f),
            "bv16": np.asarray(bv, f) * 16, "bo": np.asarray(bo, f),
        })
    _CACHED["last_in_maps"] = in_maps
    res = run_bass_kernel_spmd(nc, in_maps, core_ids=list(range(8)))
    out = np.empty((B, C, L), f)
    for c, r in enumerate(res.results):
        b, half = c // 2, c % 2
        out[b][:, half * LQ:(half + 1) * LQ] = r["outp"]
    return out
